# revision 57
# baseline (speedup 1.0000x reference)
"""Trainium2 Bass kernel for nn_AuxiliaryModel_57707180589353.

Tree-conv GNN-ish model:
  - per-leaf 1x1 conv (scalar -> C channels) + leaf node weight
  - per-unmatched-column 1x1 conv
  - 10 levels of pairwise tree merge: Conv1d(C,C,3,'same') + BN(eval) + ReLU,
    scaled by per-node weight; every level emits a [B, C, 1024] feature chunk
  - concat all chunks along length, max-pool adjacent pairs, flatten.

Sharding: data-parallel over batch B=256 across 8 cores (32 samples/core).
All parameters are tiny and replicated.

Device layout (per core): activations live as [128, 1024] bf16 SBUF tiles:
  partition p = 16*s + c  (s = sample-in-group 0..7, c = channel 0..15),
  free dim   = spatial in "split" order: col j holds position 2j (left half,
  cols 0..511) / position 2j+1 (right half, cols 512..1023).
The split order makes the conv taps contiguous matmuls and the final
pair-max-pool a dense tensor_tensor(max) of the two halves.

Conv1d(C,C,3) runs on the TensorEngine as accumulated matmuls with
block-diagonal (8 groups x 16x16) weights (BN scale folded in). Per-node
'same' zero padding: levels 1-2 hit only the valid shifted columns with
strided matmuls; levels >=3 use a full shifted matmul plus a negated-weight
fixup over the few node-boundary columns. Level 9 is a single tree node, so
its scalar node weight is folded into the conv weights/bias and its nw-mul
stage disappears. Matmuls are emitted group-major so each group's PSUM tile
closes early; the evacuation pipeline is ScalarE bias+ReLU (PSUM->bf16) ->
VectorE node-weight mul + bf16 pair-max pool -> SWDGE DMA (casting
bf16->f32 in flight on the otherwise idle GpSimd queue).

Constants arrive in three concatenated bf16 tensors (one DMA each, ordered
by first use) so the input x lands first; pooled outputs are staged two
levels per tile to halve the output-DMA count; the unmatched-column stage
is spread over levels 2-5 where the pipeline has slack.
"""

import numpy as np
import ml_dtypes

B = 256
L = 1024
U = 256
C = 16
LEVELS = 10
EPS = 1e-5
N_CORES = 8
BPC = B // N_CORES          # 32 samples per core
SPG = 8                     # samples per matmul group (8*16 = 128 partitions)
GROUPS = BPC // SPG         # 4
T_OUT = (L + U + LEVELS * L) // 2   # 5760
OUT_COLS = C * T_OUT        # 92160

BF16 = ml_dtypes.bfloat16

_CACHE = {}

# (level, group) units whose bias+ReLU runs on VectorE instead of ScalarE,
# to balance the two engines' spans.
DVE_RELU_UNITS = frozenset()

# cE1 column offsets: R | W1 | W0 | W2
E1_R, E1_W1, E1_W0, E1_W2 = 0, 512, 640, 768
E1_COLS = 896
# cE2: lwB | lbB   (leaf-stage consts, needed first)
E2_LW, E2_LB = 0, 1024
E2_COLS = 2048
# cE3: nw(level0)
E3_COLS = 1024
# cL: W0f | W1f | W2f | nw(levels 1..9) | uwB | ubB | nW0 | nW2
EL_W0F, EL_W1F, EL_W2F, EL_NW = 0, 128, 256, 384
EL_UW = 384 + 9 * L
EL_UB = EL_UW + 256
EL_NW0 = EL_UB + 256
EL_NW2 = EL_NW0 + 128
EL_COLS = EL_NW2 + 128


def _build_nc(reps=1):
    import concourse.bacc as bacc
    import concourse.tile as tile
    import concourse.mybir as mybir

    dt = mybir.dt
    f32 = dt.float32
    bf16 = dt.bfloat16
    Act = mybir.ActivationFunctionType
    Alu = mybir.AluOpType

    nc = bacc.Bacc("TRN2", target_bir_lowering=False, debug=False,
                   enable_asserts=False, num_devices=N_CORES)

    def din(name, shape, dtype=bf16):
        return nc.dram_tensor(name, list(shape), dtype, kind="ExternalInput").ap()

    x_d = din("x", [BPC, L + U])          # bf16, cast on host
    bias_d = din("bias", [128, 2], f32)   # b2P | b2P9
    cE1_d = din("cE1", [128, E1_COLS])
    cE2_d = din("cE2", [128, E2_COLS])
    cE3_d = din("cE3", [128, E3_COLS])
    cL_d = din("cL", [128, EL_COLS])
    out_d = nc.dram_tensor("out", [BPC, OUT_COLS], f32, kind="ExternalOutput").ap()

    # [4, 8, 16, 5760] view of the output: (group, sample, channel, pooled col)
    out_v = out_d.rearrange("(g s) (c t) -> g s c t", g=GROUPS, c=C)

    with tile.TileContext(nc) as tc:
        with (tc.tile_pool(name="consts", bufs=1) as cpool,
              tc.tile_pool(name="work", bufs=2) as work,
              tc.tile_pool(name="curp", bufs=12) as curp,
              tc.tile_pool(name="poolp", bufs=16) as poolp,
              tc.tile_pool(name="psp", bufs=4, space="PSUM") as psp):
            # ---- input + constant loads, in order of first use ----
            xb0 = work.tile([BPC, L + U], bf16, tag="xb", bufs=2, name="xb0")
            nc.scalar.dma_start(out=xb0, in_=x_d)
            cE1 = cpool.tile([128, E1_COLS], bf16, name="cE1")
            nc.sync.dma_start(out=cE1, in_=cE1_d)
            cE2 = cpool.tile([128, E2_COLS], bf16, name="cE2")
            nc.sync.dma_start(out=cE2, in_=cE2_d)
            cE3 = cpool.tile([128, E3_COLS], bf16, name="cE3")
            nc.sync.dma_start(out=cE3, in_=cE3_d)
            biasT = cpool.tile([128, 2], f32, name="biasT")
            nc.scalar.dma_start(out=biasT, in_=bias_d)
            cL = cpool.tile([128, EL_COLS], bf16, name="cL")
            nc.sync.dma_start(out=cL, in_=cL_d)

            R = cE1[:, E1_R:E1_R + 512]
            W1 = cE1[:, E1_W1:E1_W1 + 128]
            W0 = cE1[:, E1_W0:E1_W0 + 128]
            W2 = cE1[:, E1_W2:E1_W2 + 128]
            b2P = biasT[:, 0:1]
            b2P9 = biasT[:, 1:2]
            lwB = cE2[:, E2_LW:E2_LW + L]
            lbB = cE2[:, E2_LB:E2_LB + L]
            uwB = cL[:, EL_UW:EL_UW + U]
            ubB = cL[:, EL_UB:EL_UB + U]
            nW0 = cL[:, EL_NW0:EL_NW0 + 128]
            nW2 = cL[:, EL_NW2:EL_NW2 + 128]
            W0f = cL[:, EL_W0F:EL_W0F + 128]
            W1f = cL[:, EL_W1F:EL_W1F + 128]
            W2f = cL[:, EL_W2F:EL_W2F + 128]

            def nw_of(k):
                if k == 0:
                    return cE3[:, 0:L]
                return cL[:, EL_NW + (k - 1) * L:EL_NW + k * L]

            def mm(out, lhsT, rhs, start, stop):
                nc.tensor.matmul(out, lhsT, rhs, start=start, stop=stop,
                                 skip_group_check=True)

            for _rep in range(reps):
              if _rep == 0:
                  xb = xb0
              else:
                  xb = work.tile([BPC, L + U], bf16, tag="xb", bufs=2,
                                 name=f"xb{_rep}")
                  nc.sync.dma_start(out=xb, in_=x_d)

              # ---- leaf + unmatched: 1x1 convs via broadcast matmul ----
              curs = []
              plus = []
              for g in range(GROUPS):
                  Rg = R[0:32, g * 128:(g + 1) * 128]
                  ps = psp.tile([128, L], f32, tag="ps", name=f"psleaf{g}")
                  mm(ps[:, 0:512], Rg, xb[:, 0:L:2], True, True)
                  mm(ps[:, 512:1024], Rg, xb[:, 1:L:2], True, True)
                  cur = curp.tile([128, L], bf16, tag="cur", name=f"curleaf{g}")
                  if g < 2:
                      nc.vector.tensor_mul(out=cur, in0=ps, in1=lwB)
                  else:
                      tmp = work.tile([128, L], bf16, tag="tmp",
                                      name=f"tmpleaf{g}")
                      nc.scalar.activation(out=tmp, in_=ps, func=Act.Copy,
                                           scale=1.0)
                      nc.vector.tensor_mul(out=cur, in0=tmp, in1=lwB)
                  nc.vector.tensor_add(out=cur, in0=cur, in1=lbB)
                  plu = poolp.tile([128, 640], bf16, tag="plu", bufs=4,
                                   name=f"plu{g}")
                  nc.vector.tensor_tensor(out=plu[:, 0:512], in0=cur[:, 0:512],
                                          in1=cur[:, 512:1024], op=Alu.max)
                  curs.append(cur)
                  plus.append(plu)

              # ---- tree levels (group-major; per-group PSUM closes early) ----
              pairbuf = [None] * GROUPS

              def emit_evac(k, g, ps):
                  off = 640 + 512 * k
                  nwk = nw_of(k)
                  cur = curp.tile([128, L], bf16, tag="cur",
                                  name=f"cur{k}_{g}")
                  if (k, g) in DVE_RELU_UNITS:
                      nc.vector.tensor_scalar(
                          out=cur, in0=ps, scalar1=b2P, scalar2=0.0,
                          op0=Alu.add, op1=Alu.max)
                  else:
                      nc.scalar.activation(out=cur, in_=ps, func=Act.Relu,
                                           bias=b2P, scale=1.0)
                  nc.vector.tensor_mul(out=cur, in0=cur, in1=nwk)
                  if k % 2 == 0 and k < 8:
                      pairbuf[g] = poolp.tile([128, 1024], bf16, tag="pooled",
                                              name=f"pl{k}_{g}")
                      dst = pairbuf[g][:, 0:512]
                  elif k < 8:
                      dst = pairbuf[g][:, 512:1024]
                  else:   # k == 8: own staging + own DMA
                      pairbuf[g] = poolp.tile([128, 512], bf16, tag="pl8",
                                              bufs=4, name=f"pl8_{g}")
                      dst = pairbuf[g]
                  nc.vector.tensor_tensor(out=dst, in0=cur[:, 0:512],
                                          in1=cur[:, 512:1024], op=Alu.max)
                  if k % 2 == 1 and k < 8:
                      nc.gpsimd.dma_start(
                          out=out_v[g, :, :, off - 512:off + 512],
                          in_=pairbuf[g])
                  elif k == 8:
                      nc.gpsimd.dma_start(
                          out=out_v[g, :, :, off:off + 512],
                          in_=pairbuf[g])
                  curs[g] = cur
              for k in range(LEVELS):
                  hl = 1 << k          # node half-length in split-layout cols
                  last = (k == LEVELS - 1)
                  # level 9 is a single node: its scalar node weight is folded
                  # into the conv weights/bias, killing the nw-mul stage.
                  Wk0, Wk1, Wk2 = (W0f, W1f, W2f) if last else (W0, W1, W2)
                  bk = b2P9 if last else b2P
                  nwk = nw_of(k)
                  nfix = (512 // hl) - 1 if k > 0 else 0
                  pss = []
                  for g in range(GROUPS):
                      ce, co = curs[g][:, 0:512], curs[g][:, 512:1024]
                      ps = psp.tile([128, L], f32, tag="ps", name=f"ps{k}_{g}")
                      pe, po = ps[:, 0:512], ps[:, 512:1024]
                      mm(pe, Wk1, ce, True, False)
                      mm(po, Wk1, co, True, False)
                      if 0 < k <= 2:
                          # few valid shifted cols: hit them directly with
                          # strided matmuls instead of full-shift + fixup
                          for r in range(1, hl):
                              mm(pe[:, r:512:hl], Wk0,
                                 co[:, r - 1:511:hl], False, False)
                      elif k > 0:
                          mm(pe[:, 1:512], Wk0, co[:, 0:511], False, False)
                      mm(po, Wk0, ce, False, (k == 0))
                      if nfix > 0 and k > 2:
                          mm(pe[:, hl:512:hl], nW0, co[:, hl - 1:511:hl],
                             False, False)
                      mm(pe, Wk2, co, False, True)
                      if 0 < k <= 2:
                          # CoreSim's pending-zero span check over-reads
                          # strided APs by 3*(stride-1) bytes, so the last
                          # strided column of the po bank is patched with a
                          # 1-wide contiguous matmul instead.
                          for r in range(0, hl - 1):
                              cols = list(range(r, 511, hl))
                              lastc = cols[-1]
                              bogus = (512 + r) * 4 + hl * (4 * len(cols) - 1) + 1
                              if bogus > 4096:
                                  mm(po[:, r:lastc:hl], Wk2,
                                     ce[:, r + 1:lastc + 1:hl],
                                     False, False)
                                  mm(po[:, lastc:lastc + 1], Wk2,
                                     ce[:, lastc + 1:lastc + 2],
                                     False, r == hl - 2)
                              else:
                                  mm(po[:, r:511:hl], Wk2,
                                     ce[:, r + 1:512:hl], False, r == hl - 2)
                      elif k > 0:
                          mm(po[:, 0:511], Wk2, ce[:, 1:512], False, nfix == 0)
                      if nfix > 0 and k > 2:
                          mm(po[:, hl - 1:511:hl], nW2, ce[:, hl:512:hl],
                             False, True)
                      pss.append(ps)
                      if k < LEVELS - 1:
                          emit_evac(k, g, ps)
                  if 2 <= k <= 5:
                      for g in [k - 2]:
                          Rg = R[0:32, g * 128:(g + 1) * 128]
                          psu = psp.tile([128, U], f32, tag="ps",
                                         name=f"psunm{g}")
                          mm(psu[:, 0:128], Rg, xb[:, L:L + U:2], True, True)
                          mm(psu[:, 128:256], Rg, xb[:, L + 1:L + U:2],
                             True, True)
                          tmpu = work.tile([128, U], bf16, tag="tmpu", bufs=2,
                                           name=f"tmpunm{g}")
                          nc.scalar.activation(out=tmpu, in_=psu,
                                               func=Act.Copy, scale=1.0)
                          nc.vector.tensor_mul(out=tmpu, in0=tmpu, in1=uwB)
                          nc.vector.tensor_add(out=tmpu, in0=tmpu, in1=ubB)
                          nc.vector.tensor_tensor(out=plus[g][:, 512:640],
                                                  in0=tmpu[:, 0:128],
                                                  in1=tmpu[:, 128:256],
                                                  op=Alu.max)
                          nc.gpsimd.dma_start(out=out_v[g, :, :, 0:640],
                                              in_=plus[g])
                  off = 640 + 512 * k
                  if last:
                      # short tail: relu both halves (ACT/DVE), max, DMA
                      for g in range(GROUPS):
                          tmp9 = work.tile([128, L], bf16, tag="tmp9", bufs=4,
                                           name=f"tmp9_{g}")
                          nc.scalar.activation(out=tmp9[:, 0:512],
                                               in_=pss[g][:, 0:512],
                                               func=Act.Relu, bias=bk,
                                               scale=1.0)
                          nc.vector.tensor_scalar(
                              out=tmp9[:, 512:1024], in0=pss[g][:, 512:1024],
                              scalar1=bk, scalar2=0.0,
                              op0=Alu.add, op1=Alu.max)
                          pooled = poolp.tile([128, 512], bf16, tag="pl9",
                                              bufs=4, name=f"pl9_{g}")
                          nc.vector.tensor_tensor(out=pooled,
                                                  in0=tmp9[:, 0:512],
                                                  in1=tmp9[:, 512:1024],
                                                  op=Alu.max)
                          nc.gpsimd.dma_start(out=out_v[g, :, :, off:off + 512],
                                              in_=pooled)
                      continue


    nc.compile()
    return nc


def _split_cols(a):
    """Reorder the last axis from position order to split (even|odd) order."""
    return np.concatenate([a[..., 0::2], a[..., 1::2]], axis=-1)


def _host_consts(leaf_w, leaf_b, unm_w, unm_b, conv_w, conv_b,
                 bn_gamma, bn_beta, bn_mean, bn_var, leaf_nw, internal_nw):
    f32 = np.float32

    s = (bn_gamma / np.sqrt(bn_var + EPS)).astype(f32)
    b2 = ((conv_b - bn_mean) * s + bn_beta).astype(f32)
    b2P = np.tile(b2, SPG)                            # [128]

    lw = (leaf_w * leaf_nw[:, None]).astype(f32)      # [L, C]
    lb = (leaf_b * leaf_nw[:, None]).astype(f32)

    def bcast_cols(wLC):  # [Ncols, C] -> [128, Ncols] split order
        t = np.tile(wLC.T, (SPG, 1))                  # [128, Ncols]
        return _split_cols(t)

    lwB = bcast_cols(lw)
    lbB = bcast_cols(lb)
    uwB = bcast_cols(np.asarray(unm_w, f32))
    ubB = bcast_cols(np.asarray(unm_b, f32))

    def blockdiag(w16):  # 16x16 block -> [128, 128] block-diagonal
        out = np.zeros((128, 128), f32)
        for g in range(SPG):
            out[g * C:(g + 1) * C, g * C:(g + 1) * C] = w16
        return out

    # lhsT[(g,ci),(g,co)] = conv_w[co, ci, k] * s[co]  (BN scale folded)
    Wk = [blockdiag((conv_w[:, :, k] * s[:, None]).T) for k in range(3)]
    nw9 = np.float32(internal_nw[L - 2])   # level-9 single-node weight

    # node-weight vectors per level, expanded to [1024] in split order
    nws = []
    off = 0
    for k in range(LEVELS):
        n = L >> (k + 1)
        w = np.asarray(internal_nw[off:off + n], f32)
        off += n
        expand = np.repeat(w, 1 << (k + 1))          # [1024] position order
        nws.append(_split_cols(expand))

    cE1 = np.zeros((128, E1_COLS), f32)
    for g in range(GROUPS):
        for sl in range(SPG):
            cE1[g * SPG + sl,
                E1_R + g * 128 + sl * C:E1_R + g * 128 + (sl + 1) * C] = 1.0
    cE1[:, E1_W1:E1_W1 + 128] = Wk[1]
    cE1[:, E1_W0:E1_W0 + 128] = Wk[0]
    cE1[:, E1_W2:E1_W2 + 128] = Wk[2]
    bias = np.stack([b2P, b2P * nw9], axis=1).astype(f32)   # [128, 2]

    cE2 = np.zeros((128, E2_COLS), f32)
    cE2[:, E2_LW:E2_LW + L] = lwB
    cE2[:, E2_LB:E2_LB + L] = lbB

    cE3 = np.broadcast_to(nws[0][None, :], (128, E3_COLS)).copy()

    cL = np.zeros((128, EL_COLS), f32)
    cL[:, EL_W0F:EL_W0F + 128] = Wk[0] * nw9
    cL[:, EL_W1F:EL_W1F + 128] = Wk[1] * nw9
    cL[:, EL_W2F:EL_W2F + 128] = Wk[2] * nw9
    for k in range(1, LEVELS):
        cL[:, EL_NW + (k - 1) * L:EL_NW + k * L] = nws[k][None, :]
    cL[:, EL_UW:EL_UW + U] = uwB
    cL[:, EL_UB:EL_UB + U] = ubB
    cL[:, EL_NW0:EL_NW0 + 128] = -Wk[0]
    cL[:, EL_NW2:EL_NW2 + 128] = -Wk[2]

    return {
        "bias": bias,
        "cE1": cE1.astype(BF16),
        "cE2": cE2.astype(BF16),
        "cE3": cE3.astype(BF16),
        "cL": cL.astype(BF16),
    }


def make_in_maps(inputs):
    consts = _host_consts(
        np.asarray(inputs["leaf_w"]), np.asarray(inputs["leaf_b"]),
        np.asarray(inputs["unm_w"]), np.asarray(inputs["unm_b"]),
        np.asarray(inputs["conv_w"]), np.asarray(inputs["conv_b"]),
        np.asarray(inputs["bn_gamma"]), np.asarray(inputs["bn_beta"]),
        np.asarray(inputs["bn_mean"]), np.asarray(inputs["bn_var"]),
        np.asarray(inputs["leaf_nw"]), np.asarray(inputs["internal_nw"]))
    x = np.ascontiguousarray(
        np.asarray(inputs["x"], np.float32).astype(BF16))
    in_maps = []
    for c in range(N_CORES):
        m = dict(consts)
        m["x"] = np.ascontiguousarray(x[c * BPC:(c + 1) * BPC])
        in_maps.append(m)
    return in_maps


def kernel(x, leaf_w, leaf_b, unm_w, unm_b, conv_w, conv_b,
           bn_gamma, bn_beta, bn_mean, bn_var, leaf_nw, internal_nw):
    from concourse.bass_utils import run_bass_kernel_spmd

    if "nc" not in _CACHE:
        _CACHE["nc"] = _build_nc()
    nc = _CACHE["nc"]

    in_maps = make_in_maps(dict(
        x=x, leaf_w=leaf_w, leaf_b=leaf_b, unm_w=unm_w, unm_b=unm_b,
        conv_w=conv_w, conv_b=conv_b, bn_gamma=bn_gamma, bn_beta=bn_beta,
        bn_mean=bn_mean, bn_var=bn_var, leaf_nw=leaf_nw,
        internal_nw=internal_nw))

    res = run_bass_kernel_spmd(nc, in_maps, core_ids=list(range(N_CORES)))
    out = np.concatenate([r["out"] for r in res.results], axis=0)
    return out.astype(np.float32)
